# revision 3
# baseline (speedup 1.0000x reference)
"""Entropic OT loss (CLIP-style) on 8 trn2 NeuronCores — Bass/Tile SPMD kernel.

Math (faithful to the reference's quirks):
  L = img @ txt.T                       (N=4096, D=512)
  For M1 = 1-L and M2 = 1-L.T:
    K = exp(-M/0.01);  Kinv = 1.0/K     (reference computes the reciprocal)
    5 Sinkhorn iterations:  v = b/(K.T@u);  u = Kinv@v
    P = u[:,None]*K*v[:,None]           (quirk: v indexed by ROW)
    CE = mean_i [ logsumexp_j P[i,j] - P[i,i] ]   (labels are arange)
  loss = (CE1+CE2)/2

Sharding (column shard, N/8 = 512 columns per core):
  Core c computes X = L1[:, jc] = img @ txt_c.T and Y = L2[:, jc] = txt @ img_c.T.
  K1col = exp(100*X-100)     -> rhs of the v1-update GEMV (contract all rows)
  Kinv1T = 1/K2col           -> rhs of the u1-update GEMV (Kinv1.T row-shard == 1/K2 col-shard)
  (and symmetrically for problem 2). Each GEMV produces a local 512-chunk of the
  length-4096 vector; an AllGather rebuilds the full vector for the next GEMV.
  The cross-entropy reduces with one batched AllReduce of the row-sum vectors
  plus a scalar AllReduce.

The computed loss is NaN (matching the reference bit-for-bit in the only way
that matters: exp(-M/0.01) underflows fp32, 1/K overflows to inf, and the
Sinkhorn iterations NaN-poison P; jax's log_softmax then yields NaN).

Host-side work is limited to data marshaling: dtype cast to bf16, transpose,
and per-core slicing. All FLOPs of the algorithm run on the NeuronCores.
"""

import os
import numpy as np

import concourse.bacc as bacc
import concourse.mybir as mybir
import concourse.tile as tile
from concourse.bass_utils import run_bass_kernel_spmd

F32 = mybir.dt.float32
BF16 = mybir.dt.bfloat16
AF = mybir.ActivationFunctionType
NP_BF16 = mybir.dt.np(BF16)

N = 4096          # batch
D = 512           # feature dim
NCORES = 8
S = N // NCORES   # 512 columns per core
NT = N // 128     # 32 row tiles
ND = D // 128     # 4 contraction tiles
NCH = 8           # i-chunks of 4 row-tiles each in the big matmuls
REG = 0.01
N_ITERS = 5
SCALE = 1.0 / REG         # 100.0
INV_N = 1.0 / N           # 1/4096 (exact in bf16)
HALF_INV_N = 1.0 / (2 * N)


def _build_program():
    nc = bacc.Bacc("TRN2", target_bir_lowering=False, debug=False,
                   num_devices=NCORES)

    imgT_d = nc.dram_tensor("imgT", [D, N], BF16, kind="ExternalInput").ap()
    txtT_d = nc.dram_tensor("txtT", [D, N], BF16, kind="ExternalInput").ap()
    rhsX_d = nc.dram_tensor("rhsX", [D, S], BF16, kind="ExternalInput").ap()
    rhsY_d = nc.dram_tensor("rhsY", [D, S], BF16, kind="ExternalInput").ap()
    iln_d = nc.dram_tensor("iln", [S, D], BF16, kind="ExternalInput").ap()
    tln_d = nc.dram_tensor("tln", [S, D], BF16, kind="ExternalInput").ap()
    loss_d = nc.dram_tensor("loss", [1, 1], F32, kind="ExternalOutput").ap()

    with tile.TileContext(nc) as tc:
        with (
            tc.tile_pool(name="kmat", bufs=1) as kpool,
            tc.tile_pool(name="sb", bufs=1) as sb,
            tc.tile_pool(name="win", bufs=10) as winp,
            tc.tile_pool(name="vec", bufs=2) as vec,
            tc.tile_pool(name="scr", bufs=2) as scr,
            tc.tile_pool(name="dram", bufs=2, space="DRAM") as dram,
        ):
            one_ap = nc.const_aps.tensor(1.0, (128, 1))

            # ---- constants on the ACT engine ----
            bias_m100 = sb.tile([128, 1], F32, tag="bm100")
            nc.scalar.mul(bias_m100[:], one_ap, -SCALE)

            # K matrices, column-sharded, [row-tile t at free 512t:512t+512]
            k1 = kpool.tile([128, NT * S], BF16, tag="k1")
            k2 = kpool.tile([128, NT * S], BF16, tag="k2")
            ki1 = kpool.tile([128, NT * S], BF16, tag="ki1")  # Kinv1T col = 1/K2col
            ki2 = kpool.tile([128, NT * S], BF16, tag="ki2")  # Kinv2T col = 1/K1col

            # ---- rhs (stationary side of the big matmuls) + ldiag inputs ----
            rhsX = [sb.tile([128, S], BF16, tag=f"rx{dt}", name=f"rhsX{dt}") for dt in range(ND)]
            rhsY = [sb.tile([128, S], BF16, tag=f"ry{dt}", name=f"rhsY{dt}") for dt in range(ND)]
            for dt in range(ND):
                nc.sync.dma_start(rhsX[dt][:], rhsX_d[128 * dt:128 * (dt + 1), :])
                nc.sync.dma_start(rhsY[dt][:], rhsY_d[128 * dt:128 * (dt + 1), :])

            # diag(L) for local rows: sum_d img[i,:]*txt[i,:]  -> [128, 4]
            ldiag = sb.tile([128, ND], F32, tag="ldiag")
            for q in range(ND):
                ilq = scr.tile([128, D], BF16, tag="ilq")
                tlq = scr.tile([128, D], BF16, tag="tlq")
                nc.sync.dma_start(ilq[:], iln_d[128 * q:128 * (q + 1), :])
                nc.sync.dma_start(tlq[:], tln_d[128 * q:128 * (q + 1), :])
                prod = scr.tile([128, D], F32, tag="ldprod")
                nc.vector.tensor_mul(prod[:], ilq[:], tlq[:])
                nc.vector.reduce_sum(ldiag[:, q:q + 1], prod[:],
                                     axis=mybir.AxisListType.X)
            # diag of K1 (== diag of K2): exp(100*ldiag - 100)
            kdiag = sb.tile([128, ND], F32, tag="kdiag")
            nc.scalar.activation(kdiag[:], ldiag[:], AF.Exp,
                                 bias=bias_m100[:], scale=SCALE)

            # ---- big matmuls + K production ----
            with tc.tile_pool(name="psmm", bufs=6, space="PSUM") as psmm:
                with nc.allow_low_precision("bf16 K matrices; output is NaN"):
                    for lhs_d, kt, kit in ((imgT_d, k1, ki2), (txtT_d, k2, ki1)):
                        for ic in range(NCH):
                            win = []
                            for dt in range(ND):
                                w = winp.tile([128, 512], BF16, tag="win", name=f"win_{dt}")
                                nc.sync.dma_start(
                                    w[:],
                                    lhs_d[128 * dt:128 * (dt + 1),
                                          512 * ic:512 * (ic + 1)])
                                win.append(w)
                            rhs = rhsX if kt is k1 else rhsY
                            for tin in range(4):
                                t = 4 * ic + tin
                                ps = psmm.tile([128, S], F32, tag="mm")
                                for dt in range(ND):
                                    nc.tensor.matmul(
                                        ps[:],
                                        win[dt][:, 128 * tin:128 * (tin + 1)],
                                        rhs[dt][:],
                                        start=(dt == 0), stop=(dt == ND - 1))
                                ksl = kt[:, S * t:S * (t + 1)]
                                nc.scalar.activation(ksl, ps[:], AF.Exp,
                                                     bias=bias_m100[:],
                                                     scale=SCALE)
                                # reference computes Kinv = 1.0/K
                                nc.vector.reciprocal(kit[:, S * t:S * (t + 1)], ksl)

            # ---- Sinkhorn ----
            ufull = [None, None]  # [128, 32] bf16, element [p,t] = u[128t+p]
            vfull = [None, None]
            vloc_last = [None, None]  # [1, 512] bf16 local chunks (final iter)
            uloc_last = [None, None]
            for p in range(2):
                uf = vec.tile([128, NT], BF16, tag=f"uf{p}", name=f"uf0_{p}")
                nc.vector.memset(uf[:], INV_N)
                ufull[p] = uf

            kmat = (k1, k2)
            kinvT = (ki1, ki2)
            with tc.tile_pool(name="psg", bufs=4, space="PSUM") as psg, \
                    nc.allow_low_precision("bf16 sinkhorn vectors; output is NaN"):
                for it in range(N_ITERS):
                    for p in range(2):
                        # ---- v = (1/N) / (K.T @ u) ----
                        s_ps = psg.tile([1, S], F32, tag="gemv")
                        for t in range(NT):
                            nc.tensor.matmul(
                                s_ps[:], ufull[p][:, t:t + 1],
                                kmat[p][:, S * t:S * (t + 1)],
                                start=(t == 0), stop=(t == NT - 1))
                        vrec = scr.tile([1, S], F32, tag="vrec")
                        nc.vector.reciprocal(vrec[:], s_ps[:])
                        vbf = vec.tile([1, S], BF16, tag=f"vbf{p}")
                        nc.vector.tensor_scalar_mul(vbf[:], vrec[:], INV_N)
                        # AllGather v chunks
                        vb_in = dram.tile([1, S], BF16, tag=f"vin{p}")
                        vb_out = dram.tile([NCORES, S], BF16, tag=f"vout{p}")
                        nc.gpsimd.dma_start(vb_in[:], vbf[:])
                        nc.gpsimd.collective_compute(
                            "AllGather", mybir.AluOpType.bypass,
                            ins=[vb_in[:].opt()], outs=[vb_out[:].opt()],
                            replica_groups=[list(range(NCORES))])
                        vf = vec.tile([128, NT], BF16, tag=f"vf{p}", name=f"vf_{p}_{it}")
                        nc.sync.dma_start(
                            vf[:],
                            vb_out[:].rearrange("a b -> (a b)")
                                     .rearrange("(t q) -> q t", q=128))
                        vfull[p] = vf

                        # ---- u = Kinv @ v ----
                        u_ps = psg.tile([1, S], F32, tag="gemv")
                        for t in range(NT):
                            nc.tensor.matmul(
                                u_ps[:], vf[:, t:t + 1],
                                kinvT[p][:, S * t:S * (t + 1)],
                                start=(t == 0), stop=(t == NT - 1))
                        ubf = vec.tile([1, S], BF16, tag=f"ubf{p}")
                        nc.scalar.copy(ubf[:], u_ps[:])
                        ub_in = dram.tile([1, S], BF16, tag=f"uin{p}")
                        ub_out = dram.tile([NCORES, S], BF16, tag=f"uout{p}")
                        nc.gpsimd.dma_start(ub_in[:], ubf[:])
                        nc.gpsimd.collective_compute(
                            "AllGather", mybir.AluOpType.bypass,
                            ins=[ub_in[:].opt()], outs=[ub_out[:].opt()],
                            replica_groups=[list(range(NCORES))])
                        uf = vec.tile([128, NT], BF16, tag=f"uf{p}", name=f"uf_{p}_{it}")
                        nc.sync.dma_start(
                            uf[:],
                            ub_out[:].rearrange("a b -> (a b)")
                                     .rearrange("(t q) -> q t", q=128))
                        ufull[p] = uf
                        if it == N_ITERS - 1:
                            vloc_last[p] = vbf
                            uloc_last[p] = ubf

            # ---- loss ----
            # row sums s[i] = sum_j exp(P[i,j]); P = (u*v)[i] * K[i,j]
            s_acc = [None, None]
            with nc.allow_low_precision("bf16 P tiles; output is NaN"):
                for p in range(2):
                    cfull = sb.tile([128, NT], F32, tag=f"cf{p}")
                    nc.vector.tensor_mul(cfull[:], ufull[p][:], vfull[p][:])
                    sa = sb.tile([128, NT], F32, tag=f"sa{p}")
                    s_acc[p] = sa
                    for t in range(NT):
                        pt = scr.tile([128, S], BF16, tag="pt")
                        nc.vector.tensor_scalar_mul(
                            pt[:], kmat[p][:, S * t:S * (t + 1)],
                            cfull[:, t:t + 1])
                        ptrash = scr.tile([128, S], BF16, tag="ptrash")
                        nc.scalar.activation(ptrash[:], pt[:], AF.Exp,
                                             accum_out=sa[:, t:t + 1])

            # batched AllReduce of both problems' row-sum vectors
            sar_in = dram.tile([2, N], F32, tag="sarin")
            sar_out = dram.tile([2, N], F32, tag="sarout")
            for p in range(2):
                nc.sync.dma_start(
                    sar_in[p, :].rearrange("(t q) -> q t", q=128), s_acc[p][:])
            nc.gpsimd.collective_compute(
                "AllReduce", mybir.AluOpType.add,
                ins=[sar_in[:].opt()], outs=[sar_out[:].opt()],
                replica_groups=[list(range(NCORES))])

            total = sb.tile([128, 1], F32, tag="total")
            nc.vector.memset(total[:], 0.0)
            for p in range(2):
                sfull = scr.tile([128, NT], F32, tag="sfull")
                nc.sync.dma_start(
                    sfull[:], sar_out[p, :].rearrange("(t q) -> q t", q=128))
                logs = scr.tile([128, NT], F32, tag="logs")
                nc.scalar.activation(logs[:], sfull[:], AF.Ln)
                red = scr.tile([128, 1], F32, tag="red")
                nc.vector.reduce_sum(red[:], logs[:], axis=mybir.AxisListType.X)
                # every core computes the identical full sum; the final scalar
                # AllReduce adds 8 copies, so scale by 1/8 here
                sc = scr.tile([128, 1], F32, tag="sc")
                nc.vector.tensor_scalar_mul(sc[:], red[:], 1.0 / NCORES)
                nc.vector.tensor_add(total[:], total[:], sc[:])

                # diagonal term: P[i,i] = u[i]*v[i]*Kdiag[i] for the local rows
                dd = dram.tile([1, S], BF16, tag=f"dd{p}")
                du = dram.tile([1, S], BF16, tag=f"du{p}")
                nc.gpsimd.dma_start(dd[:], vloc_last[p][:])
                nc.gpsimd.dma_start(du[:], uloc_last[p][:])
                v128 = scr.tile([128, ND], BF16, tag="v128")
                u128 = scr.tile([128, ND], BF16, tag="u128")
                nc.sync.dma_start(
                    v128[:], dd[:].rearrange("a b -> (a b)")
                                  .rearrange("(t q) -> q t", q=128))
                nc.sync.dma_start(
                    u128[:], du[:].rearrange("a b -> (a b)")
                                  .rearrange("(t q) -> q t", q=128))
                cd = scr.tile([128, ND], F32, tag="cd")
                nc.vector.tensor_mul(cd[:], u128[:], v128[:])
                dt_ = scr.tile([128, ND], F32, tag="dt")
                nc.vector.tensor_mul(dt_[:], cd[:], kdiag[:])
                redd = scr.tile([128, 1], F32, tag="redd")
                nc.vector.reduce_sum(redd[:], dt_[:], axis=mybir.AxisListType.X)
                nc.vector.tensor_sub(total[:], total[:], redd[:])

            # partition sum via ones.T @ total (fp32 matmul, 1 column)
            with tc.tile_pool(name="pssc", bufs=1, space="PSUM") as pssc:
                tot_ps = pssc.tile([1, 1], F32, tag="tot")
                nc.tensor.matmul(tot_ps[:], one_ap, total[:],
                                 start=True, stop=True)
                tot_sb = sb.tile([1, 1], F32, tag="totsb")
                nc.scalar.copy(tot_sb[:], tot_ps[:])

            tar_in = dram.tile([1, 1], F32, tag="tarin")
            tar_out = dram.tile([1, 1], F32, tag="tarout")
            nc.gpsimd.dma_start(tar_in[:], tot_sb[:])
            nc.gpsimd.collective_compute(
                "AllReduce", mybir.AluOpType.add,
                ins=[tar_in[:].opt()], outs=[tar_out[:].opt()],
                replica_groups=[list(range(NCORES))])
            fin = sb.tile([1, 1], F32, tag="fin")
            nc.sync.dma_start(fin[:], tar_out[:])
            out_sb = sb.tile([1, 1], F32, tag="outsb")
            nc.scalar.mul(out_sb[:], fin[:], HALF_INV_N)
            nc.sync.dma_start(loss_d, out_sb[:])

    nc.compile()
    return nc


_NC_CACHE = {}


def _get_program():
    if "nc" not in _NC_CACHE:
        _NC_CACHE["nc"] = _build_program()
    return _NC_CACHE["nc"]


def kernel(all_image_features, all_text_features, labels=None, **_unused):
    img = np.asarray(all_image_features, dtype=np.float32)
    txt = np.asarray(all_text_features, dtype=np.float32)
    assert img.shape == (N, D) and txt.shape == (N, D)

    # host-side marshaling only: bf16 cast + transpose + per-core slicing
    imgT = np.ascontiguousarray(img.T).astype(NP_BF16)
    txtT = np.ascontiguousarray(txt.T).astype(NP_BF16)
    img_bf = img.astype(NP_BF16)
    txt_bf = txt.astype(NP_BF16)

    in_maps = []
    for c in range(NCORES):
        sl = slice(S * c, S * (c + 1))
        in_maps.append({
            "imgT": imgT,
            "txtT": txtT,
            "rhsX": np.ascontiguousarray(txtT[:, sl]),
            "rhsY": np.ascontiguousarray(imgT[:, sl]),
            "iln": np.ascontiguousarray(img_bf[sl, :]),
            "tln": np.ascontiguousarray(txt_bf[sl, :]),
        })

    nc = _get_program()
    trace = bool(int(os.environ.get("OT_KERNEL_TRACE", "0")))
    res = run_bass_kernel_spmd(nc, in_maps, list(range(NCORES)), trace=trace)
    if trace:
        _NC_CACHE["last_exec_time_ns"] = res.exec_time_ns
        _NC_CACHE["last_results"] = res
    loss = np.float32(res.results[0]["loss"][0, 0])
    return np.asarray(loss, dtype=np.float32).reshape(())


# revision 8
# speedup vs baseline: 1.3187x; 1.3187x over previous
"""Entropic OT loss (CLIP-style) on 8 trn2 NeuronCores — Bass/Tile SPMD kernel.

Math (faithful to the reference's quirks):
  L = img @ txt.T                       (N=4096, D=512)
  For M1 = 1-L and M2 = 1-L.T:
    K = exp(-M/0.01);  Kinv = 1.0/K     (reference computes the reciprocal)
    5 Sinkhorn iterations:  v = b/(K.T@u);  u = Kinv@v
    P = u[:,None]*K*v[:,None]           (quirk: v indexed by ROW)
    CE = mean_i [ logsumexp_j P[i,j] - P[i,i] ]   (labels are arange)
  loss = (CE1+CE2)/2

Sharding (column shard, N/8 = 512 columns per core):
  Core c computes X = L1[:, jc] = img @ txt_c.T and Y = L2[:, jc] = txt @ img_c.T.
  K1col = exp(100*X-100)     -> rhs of the v1-update GEMV (contract all rows)
  Kinv1T = 1/K2col           -> rhs of the u1-update GEMV (Kinv1.T row-shard == 1/K2 col-shard)
  (and symmetrically for problem 2). Each GEMV produces a local 512-chunk of the
  length-4096 vector; an AllGather rebuilds the full vector for the next GEMV.
  The cross-entropy reduces with one batched AllReduce of the row-sum vectors
  plus a scalar AllReduce.

The computed loss is NaN (matching the reference bit-for-bit in the only way
that matters: exp(-M/0.01) underflows fp32, 1/K overflows to inf, and the
Sinkhorn iterations NaN-poison P; jax's log_softmax then yields NaN).

Host-side work is limited to data marshaling: dtype cast to bf16, transpose,
and per-core slicing. All FLOPs of the algorithm run on the NeuronCores.
"""

import os
import numpy as np

import concourse.bacc as bacc
import concourse.mybir as mybir
import concourse.tile as tile
from concourse.bass_utils import run_bass_kernel_spmd

F32 = mybir.dt.float32
BF16 = mybir.dt.bfloat16
AF = mybir.ActivationFunctionType
NP_BF16 = mybir.dt.np(BF16)

N = 4096          # batch
D = 512           # feature dim
NCORES = 8
S = N // NCORES   # 512 columns per core
NT = N // 128     # 32 row tiles
ND = D // 128     # 4 contraction tiles
NCH = 8           # i-chunks of 4 row-tiles each in the big matmuls
REG = 0.01
N_ITERS = 5
SCALE = 1.0 / REG         # 100.0
INV_N = 1.0 / N           # 1/4096 (exact in bf16)
HALF_INV_N = 1.0 / (2 * N)


def _build_program():
    nc = bacc.Bacc("TRN2", target_bir_lowering=False, debug=False,
                   num_devices=NCORES)

    imgT_d = nc.dram_tensor("imgT", [D, N], BF16, kind="ExternalInput").ap()
    txtT_d = nc.dram_tensor("txtT", [D, N], BF16, kind="ExternalInput").ap()
    rhsX_d = nc.dram_tensor("rhsX", [D, S], BF16, kind="ExternalInput").ap()
    rhsY_d = nc.dram_tensor("rhsY", [D, S], BF16, kind="ExternalInput").ap()
    iln_d = nc.dram_tensor("iln", [S, D], BF16, kind="ExternalInput").ap()
    tln_d = nc.dram_tensor("tln", [S, D], BF16, kind="ExternalInput").ap()
    loss_d = nc.dram_tensor("loss", [1, 1], F32, kind="ExternalOutput").ap()

    with tile.TileContext(nc) as tc:
        with (
            tc.tile_pool(name="kmat", bufs=1) as kpool,
            tc.tile_pool(name="sb", bufs=1) as sb,
            tc.tile_pool(name="win", bufs=10) as winp,
            tc.tile_pool(name="vec", bufs=2) as vec,
            tc.tile_pool(name="scr", bufs=2) as scr,
            tc.tile_pool(name="dram", bufs=2, space="DRAM") as dram,
        ):
            one_ap = nc.const_aps.tensor(1.0, (128, 1))

            # ---- constants on the ACT engine ----
            bias_m100 = sb.tile([128, 1], F32, tag="bm100")
            nc.scalar.mul(bias_m100[:], one_ap, -SCALE)

            # K matrices, column-sharded, [row-tile t at free 512t:512t+512]
            k1 = kpool.tile([128, NT * S], BF16, tag="k1")
            k2 = kpool.tile([128, NT * S], BF16, tag="k2")
            ki1 = kpool.tile([128, NT * S], BF16, tag="ki1")  # Kinv1T col = 1/K2col
            ki2 = kpool.tile([128, NT * S], BF16, tag="ki2")  # Kinv2T col = 1/K1col

            # ---- rhs (stationary side of the big matmuls) + ldiag inputs ----
            rhsX = [sb.tile([128, S], BF16, tag=f"rx{dt}", name=f"rhsX{dt}") for dt in range(ND)]
            rhsY = [sb.tile([128, S], BF16, tag=f"ry{dt}", name=f"rhsY{dt}") for dt in range(ND)]
            for dt in range(ND):
                nc.sync.dma_start(rhsX[dt][:], rhsX_d[128 * dt:128 * (dt + 1), :])
                nc.sync.dma_start(rhsY[dt][:], rhsY_d[128 * dt:128 * (dt + 1), :])

            # diag(L) for local rows: sum_d img[i,:]*txt[i,:]  -> [128, 4]
            ldiag = sb.tile([128, ND], F32, tag="ldiag")
            for q in range(ND):
                ilq = scr.tile([128, D], BF16, tag="ilq")
                tlq = scr.tile([128, D], BF16, tag="tlq")
                nc.sync.dma_start(ilq[:], iln_d[128 * q:128 * (q + 1), :])
                nc.sync.dma_start(tlq[:], tln_d[128 * q:128 * (q + 1), :])
                prod = scr.tile([128, D], F32, tag="ldprod")
                nc.vector.tensor_mul(prod[:], ilq[:], tlq[:])
                nc.vector.reduce_sum(ldiag[:, q:q + 1], prod[:],
                                     axis=mybir.AxisListType.X)
            # diag of K1 (== diag of K2): exp(100*ldiag - 100)
            kdiag = sb.tile([128, ND], F32, tag="kdiag")
            nc.scalar.activation(kdiag[:], ldiag[:], AF.Exp,
                                 bias=bias_m100[:], scale=SCALE)

            # ---- big matmuls + K production ----
            # bias for Kinv = exp(100 - 100*L)  (== 1/K up to fp rounding; the
            # reference's 1.0/K overflows to the same inf/0 garbage classes)
            bias_p100 = sb.tile([128, 1], F32, tag="bp100")
            nc.scalar.mul(bias_p100[:], one_ap, SCALE)
            with tc.tile_pool(name="psmm", bufs=2, space="PSUM") as psmm:
                with nc.allow_low_precision("bf16 K matrices; output is NaN"):
                    for lhs_d, kt, kit in ((imgT_d, k1, ki2), (txtT_d, k2, ki1)):
                        for ic in range(NCH):
                            win = []
                            for dt in range(ND):
                                w = winp.tile([128, 512], BF16, tag="win", name=f"win_{dt}")
                                nc.sync.dma_start(
                                    w[:],
                                    lhs_d[128 * dt:128 * (dt + 1),
                                          512 * ic:512 * (ic + 1)])
                                win.append(w)
                            rhs = rhsX if kt is k1 else rhsY
                            # one 4-bank PSUM tile holds the whole i-chunk so
                            # the exps amortize ACT per-op overhead over 2048
                            ps = psmm.tile([128, 4 * S], F32, tag="mm")
                            for tin in range(4):
                                for dt in range(ND):
                                    nc.tensor.matmul(
                                        ps[:, S * tin:S * (tin + 1)],
                                        win[dt][:, 128 * tin:128 * (tin + 1)],
                                        rhs[dt][:],
                                        start=(dt == 0), stop=(dt == ND - 1))
                            csl = slice(S * 4 * ic, S * 4 * (ic + 1))
                            nc.scalar.activation(kt[:, csl], ps[:], AF.Exp,
                                                 bias=bias_m100[:], scale=SCALE)
                            nc.scalar.activation(kit[:, csl], ps[:], AF.Exp,
                                                 bias=bias_p100[:], scale=-SCALE)

            # ---- Sinkhorn ----
            ufull = [None, None]  # [128, 32] bf16, element [p,t] = u[128t+p]
            vfull = [None, None]
            for p in range(2):
                uf = vec.tile([128, NT], BF16, tag=f"uf{p}", name=f"uf0_{p}")
                nc.vector.memset(uf[:], INV_N)
                ufull[p] = uf

            kmat = (k1, k2)
            kinvT = (ki1, ki2)
            sb_in_last = [None, None]
            ub_in_last = [None, None]
            with tc.tile_pool(name="psg", bufs=4, space="PSUM") as psg, \
                    nc.allow_low_precision("bf16 sinkhorn vectors; output is NaN"):
                for it in range(N_ITERS):
                    # ---- s = K.T @ u  (both problems back-to-back on PE so the
                    # other problem's GEMV hides this one's AllGather) ----
                    for p in range(2):
                        s_ps = psg.tile([1, S], F32, tag="gemv",
                                        name=f"sps_{p}_{it}")
                        for t in range(NT):
                            nc.tensor.matmul(
                                s_ps[:], ufull[p][:, t:t + 1],
                                kmat[p][:, S * t:S * (t + 1)],
                                start=(t == 0), stop=(t == NT - 1))
                        s_sb = scr.tile([1, S], F32, tag=f"ssb{p}",
                                        name=f"ssb_{p}_{it}")
                        nc.scalar.copy(s_sb[:], s_ps[:])
                        sb_in = dram.tile([1, S], F32, tag=f"sin{p}",
                                          name=f"sin_{p}_{it}")
                        sb_out = dram.tile([NCORES, S], F32, tag=f"sout{p}",
                                           name=f"sout_{p}_{it}")
                        nc.sync.dma_start(sb_in[:], s_sb[:])
                        nc.gpsimd.collective_compute(
                            "AllGather", mybir.AluOpType.bypass,
                            ins=[sb_in[:].opt()], outs=[sb_out[:].opt()],
                            replica_groups=[list(range(NCORES))])
                        sf = vec.tile([128, NT], F32, tag=f"sf{p}",
                                      name=f"sf_{p}_{it}")
                        nc.sync.dma_start(
                            sf[:],
                            sb_out[:].rearrange("a b -> (a b)")
                                     .rearrange("(t q) -> q t", q=128))
                        # v = (1/N) * 1/s on all 128 lanes (cheap post-gather)
                        vr = vec.tile([128, NT], F32, tag=f"vr{p}",
                                      name=f"vr_{p}_{it}")
                        nc.vector.reciprocal(vr[:], sf[:])
                        vf = vec.tile([128, NT], BF16, tag=f"vf{p}",
                                      name=f"vf_{p}_{it}")
                        nc.vector.tensor_scalar_mul(vf[:], vr[:], INV_N)
                        vfull[p] = vf
                        if it == N_ITERS - 1:
                            sb_in_last[p] = sb_in
                    # ---- u = Kinv @ v ----
                    for p in range(2):
                        u_ps = psg.tile([1, S], F32, tag="gemv",
                                        name=f"ups_{p}_{it}")
                        for t in range(NT):
                            nc.tensor.matmul(
                                u_ps[:], vfull[p][:, t:t + 1],
                                kinvT[p][:, S * t:S * (t + 1)],
                                start=(t == 0), stop=(t == NT - 1))
                        u_sb = scr.tile([1, S], F32, tag=f"usb{p}",
                                        name=f"usb_{p}_{it}")
                        nc.scalar.copy(u_sb[:], u_ps[:])
                        ub_in = dram.tile([1, S], F32, tag=f"uin{p}",
                                          name=f"uin_{p}_{it}")
                        ub_out = dram.tile([NCORES, S], F32, tag=f"uout{p}",
                                           name=f"uout_{p}_{it}")
                        nc.sync.dma_start(ub_in[:], u_sb[:])
                        nc.gpsimd.collective_compute(
                            "AllGather", mybir.AluOpType.bypass,
                            ins=[ub_in[:].opt()], outs=[ub_out[:].opt()],
                            replica_groups=[list(range(NCORES))])
                        uff = vec.tile([128, NT], F32, tag=f"uff{p}",
                                       name=f"uff_{p}_{it}")
                        nc.sync.dma_start(
                            uff[:],
                            ub_out[:].rearrange("a b -> (a b)")
                                     .rearrange("(t q) -> q t", q=128))
                        uf = vec.tile([128, NT], BF16, tag=f"uf{p}",
                                      name=f"uf_{p}_{it}")
                        nc.vector.tensor_copy(uf[:], uff[:])
                        ufull[p] = uf
                        if it == N_ITERS - 1:
                            ub_in_last[p] = ub_in

            # ---- loss ----
            # row sums s[i] = sum_j exp(P[i,j]); P = (u*v)[i] * K[i,j]
            s_acc = [None, None]
            with nc.allow_low_precision("bf16 P tiles; output is NaN"):
                for p in range(2):
                    cfull = sb.tile([128, NT], F32, tag=f"cf{p}")
                    nc.vector.tensor_mul(cfull[:], ufull[p][:], vfull[p][:])
                    sa = sb.tile([128, NT], F32, tag=f"sa{p}")
                    s_acc[p] = sa
                    for t in range(NT):
                        pt = scr.tile([128, S], BF16, tag="pt")
                        nc.vector.tensor_scalar_mul(
                            pt[:], kmat[p][:, S * t:S * (t + 1)],
                            cfull[:, t:t + 1])
                        ptrash = scr.tile([128, S], BF16, tag="ptrash")
                        nc.scalar.activation(ptrash[:], pt[:], AF.Exp,
                                             accum_out=sa[:, t:t + 1])

            # batched AllReduce of both problems' row-sum vectors
            sar_in = dram.tile([2, N], F32, tag="sarin")
            sar_out = dram.tile([2, N], F32, tag="sarout")
            for p in range(2):
                nc.sync.dma_start(
                    sar_in[p, :].rearrange("(t q) -> q t", q=128), s_acc[p][:])
            nc.gpsimd.collective_compute(
                "AllReduce", mybir.AluOpType.add,
                ins=[sar_in[:].opt()], outs=[sar_out[:].opt()],
                replica_groups=[list(range(NCORES))])

            total = sb.tile([128, 1], F32, tag="total")
            nc.vector.memset(total[:], 0.0)
            for p in range(2):
                sfull = scr.tile([128, NT], F32, tag="sfull")
                nc.sync.dma_start(
                    sfull[:], sar_out[p, :].rearrange("(t q) -> q t", q=128))
                logs = scr.tile([128, NT], F32, tag="logs")
                nc.scalar.activation(logs[:], sfull[:], AF.Ln)
                red = scr.tile([128, 1], F32, tag="red")
                nc.vector.reduce_sum(red[:], logs[:], axis=mybir.AxisListType.X)
                # every core computes the identical full sum; the final scalar
                # AllReduce adds 8 copies, so scale by 1/8 here
                sc = scr.tile([128, 1], F32, tag="sc")
                nc.vector.tensor_scalar_mul(sc[:], red[:], 1.0 / NCORES)
                nc.vector.tensor_add(total[:], total[:], sc[:])

                # diagonal term: P[i,i] = u[i]*v[i]*Kdiag[i] for the local rows
                # (rebuilt from the final-iteration local chunks in DRAM)
                s128 = scr.tile([128, ND], F32, tag="s128")
                u128 = scr.tile([128, ND], F32, tag="u128")
                nc.sync.dma_start(
                    s128[:], sb_in_last[p][:].rearrange("a b -> (a b)")
                                             .rearrange("(t q) -> q t", q=128))
                nc.sync.dma_start(
                    u128[:], ub_in_last[p][:].rearrange("a b -> (a b)")
                                             .rearrange("(t q) -> q t", q=128))
                vr128 = scr.tile([128, ND], F32, tag="vr128")
                nc.vector.reciprocal(vr128[:], s128[:])
                v128 = scr.tile([128, ND], F32, tag="v128")
                nc.vector.tensor_scalar_mul(v128[:], vr128[:], INV_N)
                cd = scr.tile([128, ND], F32, tag="cd")
                nc.vector.tensor_mul(cd[:], u128[:], v128[:])
                dt_ = scr.tile([128, ND], F32, tag="dt")
                nc.vector.tensor_mul(dt_[:], cd[:], kdiag[:])
                redd = scr.tile([128, 1], F32, tag="redd")
                nc.vector.reduce_sum(redd[:], dt_[:], axis=mybir.AxisListType.X)
                nc.vector.tensor_sub(total[:], total[:], redd[:])

            # partition sum via ones.T @ total (fp32 matmul, 1 column)
            with tc.tile_pool(name="pssc", bufs=1, space="PSUM") as pssc:
                tot_ps = pssc.tile([1, 1], F32, tag="tot")
                nc.tensor.matmul(tot_ps[:], one_ap, total[:],
                                 start=True, stop=True)
                tot_sb = sb.tile([1, 1], F32, tag="totsb")
                nc.scalar.copy(tot_sb[:], tot_ps[:])

            tar_in = dram.tile([1, 1], F32, tag="tarin")
            tar_out = dram.tile([1, 1], F32, tag="tarout")
            nc.gpsimd.dma_start(tar_in[:], tot_sb[:])
            nc.gpsimd.collective_compute(
                "AllReduce", mybir.AluOpType.add,
                ins=[tar_in[:].opt()], outs=[tar_out[:].opt()],
                replica_groups=[list(range(NCORES))])
            fin = sb.tile([1, 1], F32, tag="fin")
            nc.sync.dma_start(fin[:], tar_out[:])
            out_sb = sb.tile([1, 1], F32, tag="outsb")
            nc.scalar.mul(out_sb[:], fin[:], HALF_INV_N)
            nc.sync.dma_start(loss_d, out_sb[:])

    nc.compile()
    return nc


_NC_CACHE = {}


def _get_program():
    if "nc" not in _NC_CACHE:
        _NC_CACHE["nc"] = _build_program()
    return _NC_CACHE["nc"]


def kernel(all_image_features, all_text_features, labels=None, **_unused):
    img = np.asarray(all_image_features, dtype=np.float32)
    txt = np.asarray(all_text_features, dtype=np.float32)
    assert img.shape == (N, D) and txt.shape == (N, D)

    # host-side marshaling only: bf16 cast + transpose + per-core slicing
    imgT = np.ascontiguousarray(img.T).astype(NP_BF16)
    txtT = np.ascontiguousarray(txt.T).astype(NP_BF16)
    img_bf = img.astype(NP_BF16)
    txt_bf = txt.astype(NP_BF16)

    in_maps = []
    for c in range(NCORES):
        sl = slice(S * c, S * (c + 1))
        in_maps.append({
            "imgT": imgT,
            "txtT": txtT,
            "rhsX": np.ascontiguousarray(txtT[:, sl]),
            "rhsY": np.ascontiguousarray(imgT[:, sl]),
            "iln": np.ascontiguousarray(img_bf[sl, :]),
            "tln": np.ascontiguousarray(txt_bf[sl, :]),
        })

    nc = _get_program()
    trace = bool(int(os.environ.get("OT_KERNEL_TRACE", "0")))
    res = run_bass_kernel_spmd(nc, in_maps, list(range(NCORES)), trace=trace)
    if trace:
        _NC_CACHE["last_exec_time_ns"] = res.exec_time_ns
        _NC_CACHE["last_results"] = res
    loss = np.float32(res.results[0]["loss"][0, 0])
    return np.asarray(loss, dtype=np.float32).reshape(())
